# revision 11
# baseline (speedup 1.0000x reference)
"""GNN message-passing (ConvGraph) Trainium2 Bass kernel, 8 NeuronCores.

Computes out = segment_sum(edge_weight * (x @ W)[edge_src], edge_dst) for a
graph with N nodes and E edges.

Strategy (per sharding hint):
  - Shard nodes (rows of x / out) across the 8 cores; replicate W.
  - Each core computes its h shard = x_m @ W on TensorE, then an AllGather
    makes the full h table [N_pad, 128] resident in every core's HBM.
  - Edges are partitioned by destination core on the host; per core they are
    grouped by (dst block of 128 nodes, h-table chunk of <=32k rows) and
    padded to 128-edge groups so the whole device program is static and
    identical across cores (SPMD).
  - The per-edge h rows are fetched with SWDGE dma_gather (512B rows, int16
    chunk-local indices); the weighted segment-sum is one matmul per
    128-edge group: psum[dst128, f128] += S_T[e,dst].T @ msgs[e,f], where
    S_T = (iota == dst_local) * w is built on VectorE in a single fused
    tensor_scalar op.

Host-side work is limited to sharding/layout: edge partitioning + sorting,
index conversion, and the x transpose (input staging).
"""

import os
import sys
from contextlib import ExitStack

import numpy as np

for _p in ("/opt/trn_rl_repo",):
    if _p not in sys.path and os.path.isdir(_p):
        sys.path.insert(0, _p)

import concourse.bass as bass  # noqa: E402
import concourse.mybir as mybir  # noqa: E402
import concourse.tile as tile  # noqa: E402
from concourse import bacc, library_config  # noqa: E402
from concourse.bass_utils import run_bass_kernel_spmd  # noqa: E402

N_CORES = 8
P = 128
D_IN = 256
D_OUT = 128
NCHUNK = 4  # h-table split so chunk-local indices fit int16


def make_cfg(n_nodes: int) -> dict:
    assert n_nodes % N_CORES == 0
    r0 = n_nodes // N_CORES
    r = ((r0 + P - 1) // P) * P
    nb = r // P
    sb = 1
    for cand in (7, 8, 6, 5, 4, 9, 10, 3, 2, 14, 1):
        if nb % cand == 0:
            sb = cand
            break
    ch = (N_CORES * r) // NCHUNK
    assert ch <= 32767, f"chunk rows {ch} exceed int16 index range"
    assert ch % r == 0 or r % ch == 0
    return dict(
        n_nodes=n_nodes, R0=r0, R=r, NB=nb, SB=sb, NSB=nb // sb, CH=ch
    )


ABLATE = os.environ.get("GNN_ABLATE", "")


def build_bass(cfg: dict, S: int):
    """Build the SPMD Bass program (same NEFF for all 8 cores)."""
    R, NB, SB, NSB, CH = cfg["R"], cfg["NB"], cfg["SB"], cfg["NSB"], cfg["CH"]
    NG = NB * NCHUNK * S  # total 128-edge groups per core
    NGC = SB * S  # groups per gather call
    NI = NGC * P  # idxs per gather call
    TOT = NG * P  # total padded edge slots per core
    f32 = mybir.dt.float32
    i16 = mybir.dt.int16

    nc = bacc.Bacc(
        "TRN2", target_bir_lowering=False, debug=False, num_devices=N_CORES
    )

    xT = nc.declare_dram_parameter("xT", [D_IN, R], f32, isOutput=False)
    Wp = nc.declare_dram_parameter("W", [D_IN, D_OUT], f32, isOutput=False)
    iota = nc.declare_dram_parameter("iota", [P, P], f32, isOutput=False)
    idxp = nc.declare_dram_parameter("idx", [P, TOT // 16], i16, isOutput=False)
    wgtp = nc.declare_dram_parameter("wgt", [P, NG], f32, isOutput=False)
    dstp = nc.declare_dram_parameter("dstl", [P, NG], f32, isOutput=False)
    outp = nc.declare_dram_parameter("out", [R, D_OUT], f32, isOutput=True)

    h_shard = nc.dram_tensor("h_shard", [R, D_OUT], f32)
    h_full = nc.dram_tensor(
        "h_full", [N_CORES * R, D_OUT], f32, addr_space="Shared"
    )

    DK = D_IN // P  # k-chunks for the projection matmul

    with tile.TileContext(nc) as tc, ExitStack() as ctx:
        const = ctx.enter_context(tc.tile_pool(name="const", bufs=1))
        xpool = ctx.enter_context(tc.tile_pool(name="xp", bufs=3))
        hstage = ctx.enter_context(tc.tile_pool(name="hst", bufs=3))
        psum = ctx.enter_context(tc.tile_pool(name="ps", bufs=8, space="PSUM"))
        gpool = ctx.enter_context(tc.tile_pool(name="gat", bufs=2))
        ipool = ctx.enter_context(tc.tile_pool(name="idxp", bufs=2))
        mpool = ctx.enter_context(tc.tile_pool(name="meta", bufs=4))
        spool = ctx.enter_context(tc.tile_pool(name="oneh", bufs=4))
        opool = ctx.enter_context(tc.tile_pool(name="ost", bufs=2))

        nc.gpsimd.load_library(library_config.mlp)

        w_t = const.tile([P, DK, P], f32)
        for k in range(DK):
            nc.sync.dma_start(out=w_t[:, k, :], in_=Wp[k * P : (k + 1) * P, :])
        iota_t = const.tile([P, P], f32)
        nc.sync.dma_start(out=iota_t[:], in_=iota[:])

        # Phase A: h_shard = x_m @ W
        TS = 8  # row-tiles per strip
        nstrip = (NB + TS - 1) // TS
        for s_ in range(nstrip):
            t0 = s_ * TS
            t1 = min(NB, t0 + TS)
            nt = t1 - t0
            xk = []
            for k in range(DK):
                xkt = xpool.tile([P, TS * P], f32, tag=f"x{k}")
                nc.sync.dma_start(
                    out=xkt[:, : nt * P],
                    in_=xT[k * P : (k + 1) * P, t0 * P : t1 * P],
                )
                xk.append(xkt)
            hst = hstage.tile([P, TS, P], f32, tag="hst")
            for t in range(nt):
                ps = psum.tile([P, P], f32, tag="ps")
                for k in range(DK):
                    nc.tensor.matmul(
                        ps[:],
                        xk[k][:, t * P : (t + 1) * P],
                        w_t[:, k, :],
                        start=(k == 0),
                        stop=(k == DK - 1),
                    )
                nc.vector.tensor_copy(out=hst[:, t, :], in_=ps[:])
            nc.sync.dma_start(
                out=h_shard[t0 * P : t1 * P, :].rearrange(
                    "(t p) f -> p t f", p=P
                ),
                in_=hst[:, :nt, :],
            )

        # Phase B: AllGather h across the 8 cores
        nc.gpsimd.collective_compute(
            "AllGather",
            mybir.AluOpType.bypass,
            ins=[h_shard[:]],
            outs=[h_full[:]],
            replica_groups=[list(range(N_CORES))],
        )

        # Phase C: gather + weighted segment-sum
        for sb in range(NSB if "nophasec" not in ABLATE else 0):
            ps_blocks = [
                psum.tile([P, P], f32, tag="ps", name=f"psb_{sb}_{b}")
                for b in range(SB)
            ]
            for c in range(NCHUNK):
                call = sb * NCHUNK + c
                it = ipool.tile([P, NI // 16], i16, tag="it")
                nc.sync.dma_start(
                    out=it[:],
                    in_=idxp[:, call * (NI // 16) : (call + 1) * (NI // 16)],
                )
                gt = gpool.tile([P, NGC, P], f32, tag="gt")
                if "nogather" in ABLATE:
                    nc.vector.memset(gt[:], 0.0)
                else:
                    nc.gpsimd.dma_gather(
                        gt[:],
                        h_full[c * CH : (c + 1) * CH, :],
                        it[:],
                        NI,
                        NI,
                        P,
                        single_packet=False,
                    )
                wt = mpool.tile([P, NGC], f32, tag="wt")
                dt = mpool.tile([P, NGC], f32, tag="dt")
                nc.sync.dma_start(
                    out=wt[:], in_=wgtp[:, call * NGC : (call + 1) * NGC]
                )
                nc.sync.dma_start(
                    out=dt[:], in_=dstp[:, call * NGC : (call + 1) * NGC]
                )
                for b in range(SB):
                    for j in range(S):
                        g = b * S + j
                        st = spool.tile([P, P], f32, tag="st")
                        if "nots" in ABLATE:
                            nc.vector.tensor_copy(out=st[:], in_=iota_t[:])
                        else:
                            nc.vector.tensor_scalar(
                                out=st[:],
                                in0=iota_t[:],
                                scalar1=dt[:, g : g + 1],
                                scalar2=wt[:, g : g + 1],
                                op0=mybir.AluOpType.is_equal,
                                op1=mybir.AluOpType.mult,
                            )
                        if "nomm" not in ABLATE:
                            nc.tensor.matmul(
                                ps_blocks[b][:],
                                st[:],
                                gt[:, g, :],
                                start=(c == 0 and j == 0),
                                stop=(c == NCHUNK - 1 and j == S - 1),
                            )
            ot = opool.tile([P, SB, P], f32, tag="ot")
            if "nomm" in ABLATE:
                nc.vector.memset(ot[:], 0.0)
            else:
                for b in range(SB):
                    nc.scalar.copy(out=ot[:, b, :], in_=ps_blocks[b][:])
            nc.sync.dma_start(
                out=outp[sb * SB * P : (sb + 1) * SB * P, :].rearrange(
                    "(b p) f -> p b f", p=P
                ),
                in_=ot[:],
            )

    nc.compile()
    return nc


def host_prep(x, W, edge_src, edge_dst, edge_weight, cfg):
    """Shard + stage inputs. Returns (in_maps, S)."""
    R0, R, NB, SB, CH = cfg["R0"], cfg["R"], cfg["NB"], cfg["SB"], cfg["CH"]
    x = np.asarray(x, dtype=np.float32)
    W = np.asarray(W, dtype=np.float32)
    edge_src = np.asarray(edge_src, dtype=np.int64)
    edge_dst = np.asarray(edge_dst, dtype=np.int64)
    edge_weight = np.asarray(edge_weight, dtype=np.float32)

    core_of = edge_dst // R0
    per_core = []
    max_count = 1
    for m in range(N_CORES):
        sel = core_of == m
        s = edge_src[sel]
        d = edge_dst[sel] - m * R0
        w = edge_weight[sel]
        b = d // P
        dstl = (d % P).astype(np.float32)
        hrow = (s // R0) * R + (s % R0)
        c = hrow // CH
        lidx = (hrow % CH).astype(np.int16)
        key = (b * NCHUNK + c).astype(np.int64)
        counts = np.bincount(key, minlength=NB * NCHUNK)
        max_count = max(max_count, int(counts.max()))
        per_core.append((b, c, dstl, lidx, w, key, counts))

    S = (max_count + P - 1) // P
    NG = NB * NCHUNK * S
    TOT = NG * P

    iota_np = np.tile(np.arange(P, dtype=np.float32)[None, :], (P, 1))

    in_maps = []
    for m in range(N_CORES):
        b, c, dstl, lidx, w, key, counts = per_core[m]
        order = np.argsort(key, kind="stable")
        key_s = key[order]
        starts = np.zeros(NB * NCHUNK + 1, dtype=np.int64)
        np.cumsum(counts, out=starts[1:])
        rank = np.arange(len(key_s)) - starts[key_s]
        bb = b[order]
        cc = c[order]
        slot_base = (
            (bb // SB) * (NCHUNK * SB) + cc * SB + (bb % SB)
        ) * S * P
        slot = slot_base + rank

        idx_stream = np.zeros(TOT, dtype=np.int16)
        wgt_stream = np.zeros(TOT, dtype=np.float32)
        dst_stream = np.zeros(TOT, dtype=np.float32)
        idx_stream[slot] = lidx[order]
        wgt_stream[slot] = w[order]
        dst_stream[slot] = dstl[order]

        idx_wrapped = np.ascontiguousarray(
            np.tile(idx_stream.reshape(-1, 16).T, (8, 1))
        )
        wgt_tile = np.ascontiguousarray(wgt_stream.reshape(NG, P).T)
        dst_tile = np.ascontiguousarray(dst_stream.reshape(NG, P).T)

        x_m = np.zeros((R, D_IN), dtype=np.float32)
        x_m[:R0] = x[m * R0 : (m + 1) * R0]
        xT_m = np.ascontiguousarray(x_m.T)

        in_maps.append(
            {
                "xT": xT_m,
                "W": W,
                "iota": iota_np,
                "idx": idx_wrapped,
                "wgt": wgt_tile,
                "dstl": dst_tile,
            }
        )
    return in_maps, S


_BUILD_CACHE: dict = {}


def run(x, W, edge_src, edge_dst, edge_weight, trace=False, trace_kwargs=None):
    n_nodes = x.shape[0]
    cfg = make_cfg(n_nodes)
    in_maps, S = host_prep(x, W, edge_src, edge_dst, edge_weight, cfg)
    key = (n_nodes, S)
    if key not in _BUILD_CACHE:
        _BUILD_CACHE[key] = build_bass(cfg, S)
    nc = _BUILD_CACHE[key]
    res = run_bass_kernel_spmd(
        nc,
        in_maps,
        core_ids=list(range(N_CORES)),
        trace=trace,
        **(trace_kwargs or {}),
    )
    R0, R = cfg["R0"], cfg["R"]
    out = np.concatenate(
        [np.asarray(res.results[m]["out"])[:R0] for m in range(N_CORES)], axis=0
    )
    return out, res


def kernel(**inputs) -> np.ndarray:
    out, _ = run(
        inputs["x"],
        inputs["W"],
        inputs["edge_src"],
        inputs["edge_dst"],
        inputs["edge_weight"],
        trace=False,
    )
    return out
